# revision 1
# baseline (speedup 1.0000x reference)
"""GAT (2-layer) on 8 NeuronCores — Bass/Tile kernel.

Strategy (dst-sharded graph parallel):
  - Each core owns 12500 destination nodes; per core the dsts are split into
    6 sub-shards (round-robin over the degree-sorted order). Each sub-shard's
    distinct source nodes (~28k < 32768) are renumbered by the host so edge
    gather indices fit dma_gather's int16 contract.
  - Host pre-pass is index-only: degree-sorted 128-dst tiles padded to a
    capacity grid, per-edge gather indices (wrapped int16 layout), edge pad
    masks, permutations. All model FLOPs run on device.
  - Launch A: per-core Wh1^T = (x W1 + b)^T + attention scalars s_i/s_j.
  - Host packs per-(core,sub-shard) tables: row = [Wh fp16[64] | s_j f32 | 0pad]
    (256B rows, pure repacking/indexing of device-computed values).
  - Launch B (x2, one per GAT layer): dma_gather of table rows, dense
    masked segment softmax over the slot axis, alpha-weighted message sum,
    leaky-relu, epilogue matmul with next-layer weights -> next table
    (or final fc output for layer 2).
"""

import dataclasses
import numpy as np

import concourse.bacc as bacc
import concourse.tile as tile
from concourse import bass, mybir, bass_utils
from concourse.masks import make_identity

F32 = mybir.dt.float32
F16 = mybir.dt.float16
I16 = mybir.dt.int16

N_NODES = 100000
N_CORES = 8
DPC = N_NODES // N_CORES
F = 64
IN_C = 128
NSUB = 6
RSUB = 32768  # table rows per sub-shard (int16 index range)
CAP_GRID = (4, 8, 12, 16, 20, 24, 28, 32, 40, 64, 128)
CALL_W = 64
NEG_BIG = -1.0e30
EPS = 1e-16
ALPHA = 0.2


@dataclasses.dataclass
class Schedule:
    n_tiles: int
    w_total: int
    tiles: list  # per tile: (sub, D)
    calls: list  # (sub, t0, ntc, D, col0)
    perms: list  # per core: int64 [n_tiles*128], local dst or -1
    idx16: list  # per core: int16 [128, 8*w_total]
    emask: list  # per core: f32 [128, w_total]
    flags: list  # per core: f32 [128, n_tiles]
    uniq: list  # per core: list of NSUB arrays of global node ids


def _roundup_grid(x):
    for g in CAP_GRID:
        if x <= g:
            return g
    raise ValueError(x)


def build_schedule(edge_index: np.ndarray) -> Schedule:
    src = np.asarray(edge_index[0], dtype=np.int64)
    dst = np.asarray(edge_index[1], dtype=np.int64)
    order = np.argsort(dst, kind="stable")
    src_s = src[order]
    deg_all = np.bincount(dst, minlength=N_NODES)
    starts_all = np.concatenate([[0], np.cumsum(deg_all)])

    # per-core sub-shard dst lists (round-robin over degree-sorted order)
    core_subs = []  # [core][sub] -> local dst ids
    for c in range(N_CORES):
        deg = deg_all[c * DPC : (c + 1) * DPC]
        rank = np.argsort(deg, kind="stable")
        core_subs.append([rank[s::NSUB] for s in range(NSUB)])

    # shared tile plan: per (sub, tile): D = grid(max over cores of tile max-deg)
    tiles = []
    for s in range(NSUB):
        nt = -(-max(len(core_subs[c][s]) for c in range(N_CORES)) // 128)
        for t in range(nt):
            mx = 1
            for c in range(N_CORES):
                lst = core_subs[c][s][t * 128 : (t + 1) * 128]
                if len(lst):
                    deg = deg_all[c * DPC + lst]
                    mx = max(mx, int(deg.max()))
            tiles.append((s, _roundup_grid(mx)))
    n_tiles = len(tiles)

    # call plan: consecutive same-sub same-D tiles, width <= CALL_W
    calls = []
    i = 0
    col = 0
    while i < n_tiles:
        s, D = tiles[i]
        ntc = 1
        while (
            i + ntc < n_tiles
            and tiles[i + ntc] == (s, D)
            and (ntc + 1) * D <= CALL_W
        ):
            ntc += 1
        calls.append((s, i, ntc, D, col))
        col += ntc * D
        i += ntc
    w_total = col

    perms, idx16s, emasks, flagss, uniqs = [], [], [], [], []
    for c in range(N_CORES):
        perm = np.full(n_tiles * 128, -1, dtype=np.int64)
        ti = 0
        for s in range(NSUB):
            nt = sum(1 for (ss, _) in tiles if ss == s)
            lst = core_subs[c][s]
            block = np.full(nt * 128, -1, dtype=np.int64)
            block[: len(lst)] = lst
            perm[ti * 128 : (ti + nt) * 128] = block
            ti += nt

        # per-sub unique sources + renumber map
        uniq_c = []
        remap = {}
        for s in range(NSUB):
            lst = core_subs[c][s]
            gids = c * DPC + lst
            es = np.concatenate(
                [src_s[starts_all[g] : starts_all[g + 1]] for g in gids]
            ) if len(gids) else np.array([], np.int64)
            u = np.unique(es)
            assert len(u) <= RSUB, (c, s, len(u))
            uniq_c.append(u)
            m = np.full(N_NODES, -1, np.int32)
            m[u] = np.arange(len(u), dtype=np.int32)
            remap[s] = m

        idx16 = np.zeros((128, 8 * w_total), np.int16)
        emask = np.full((128, w_total), np.float32(NEG_BIG))
        for (s, t0, ntc, D, col0) in calls:
            W = ntc * D
            idxmat = np.zeros((128, W), np.int32)
            m = remap[s]
            for tl in range(ntc):
                tglob = t0 + tl
                dsts = perm[tglob * 128 : (tglob + 1) * 128]
                for p in range(128):
                    d = dsts[p]
                    if d < 0:
                        continue
                    g = c * DPC + d
                    e0, ne = starts_all[g], deg_all[g]
                    idxmat[p, tl * D : tl * D + ne] = m[src_s[e0 : e0 + ne]]
                    emask[p, col0 + tl * D : col0 + tl * D + ne] = 0.0
            flat = idxmat.T.ravel()  # i -> (p=i%128, col=i//128)
            wrapped = np.tile(flat.reshape(-1, 16).T, (8, 1)).astype(np.int16)
            idx16[:, 8 * col0 : 8 * (col0 + W)] = wrapped
        # zero aggregation for phantom rows AND degree-0 dsts (reference
        # yields zero aggregation there; epilogue bias still applies)
        pflat = perm.copy()
        okdeg = (pflat >= 0) & (deg_all[np.clip(c * DPC + pflat, 0, N_NODES - 1)] > 0)
        flags = np.ascontiguousarray(
            okdeg.reshape(n_tiles, 128).T.astype(np.float32)
        )
        perms.append(perm)
        idx16s.append(idx16)
        emasks.append(emask)
        flagss.append(flags)
        uniqs.append(uniq_c)

    return Schedule(n_tiles, w_total, tiles, calls, perms, idx16s, emasks, flagss, uniqs)


# ---------------------------------------------------------------- prog A
def build_progA(n_loc=DPC, in_c=IN_C, f=F):
    nc = bacc.Bacc("TRN2", target_bir_lowering=False, debug=False, num_devices=N_CORES)
    xT = nc.dram_tensor("xT", [in_c, n_loc], F32, kind="ExternalInput").ap()
    W = nc.dram_tensor("W", [in_c, f], F32, kind="ExternalInput").ap()
    bW = nc.dram_tensor("bW", [f, 1], F32, kind="ExternalInput").ap()
    As = nc.dram_tensor("As", [f, 2], F32, kind="ExternalInput").ap()
    whT = nc.dram_tensor("whT", [f, n_loc], F32, kind="ExternalOutput").ap()
    s = nc.dram_tensor("s", [2, n_loc], F32, kind="ExternalOutput").ap()

    with tile.TileContext(nc) as tc:
        with tc.tile_pool(name="sb", bufs=1) as pool, tc.tile_pool(
            name="ps", bufs=2, space="PSUM"
        ) as pps, tc.tile_pool(name="sb2", bufs=3) as pool2:
            xT_sb = pool.tile([in_c, n_loc], F32)
            nc.sync.dma_start(out=xT_sb[:], in_=xT[:, :])
            W_sb = pool.tile([in_c, f], F32)
            nc.sync.dma_start(out=W_sb[:], in_=W[:, :])
            bW_sb = pool.tile([f, 1], F32)
            nc.sync.dma_start(out=bW_sb[:], in_=bW[:, :])
            As_sb = pool.tile([f, 2], F32)
            nc.sync.dma_start(out=As_sb[:], in_=As[:, :])

            CH = 512
            for c0 in range(0, n_loc, CH):
                ch = min(CH, n_loc - c0)
                ps_w = pps.tile([f, CH], F32, space="PSUM")
                nc.tensor.matmul(
                    out=ps_w[:, :ch],
                    lhsT=W_sb[:],
                    rhs=xT_sb[:, c0 : c0 + ch],
                    start=True,
                    stop=True,
                )
                wh_sb = pool2.tile([f, CH], F32, tag="wh")
                nc.scalar.activation(
                    out=wh_sb[:, :ch],
                    in_=ps_w[:, :ch],
                    func=mybir.ActivationFunctionType.Identity,
                    bias=bW_sb[:],
                )
                nc.sync.dma_start(out=whT[:, c0 : c0 + ch], in_=wh_sb[:, :ch])
                ps_s = pps.tile([2, CH], F32, space="PSUM")
                nc.tensor.matmul(
                    out=ps_s[:, :ch],
                    lhsT=As_sb[:],
                    rhs=wh_sb[:, :ch],
                    start=True,
                    stop=True,
                )
                s_sb = pool2.tile([2, CH], F32, tag="ssb")
                nc.vector.tensor_copy(out=s_sb[:, :ch], in_=ps_s[:, :ch])
                nc.sync.dma_start(out=s[:, c0 : c0 + ch], in_=s_sb[:, :ch])
    nc.compile()
    return nc


# ---------------------------------------------------------------- prog B
def build_progB(sched: Schedule, f=F):
    NT = sched.n_tiles
    WTOT = sched.w_total
    nc = bacc.Bacc("TRN2", target_bir_lowering=False, debug=False, num_devices=N_CORES)
    table = nc.dram_tensor("table", [NSUB * RSUB, f], F32, kind="ExternalInput").ap()
    idx_d = nc.dram_tensor("idx", [128, 8 * WTOT], I16, kind="ExternalInput").ap()
    em_d = nc.dram_tensor("emask", [128, WTOT], F32, kind="ExternalInput").ap()
    si_d = nc.dram_tensor("si", [128, NT], F32, kind="ExternalInput").ap()
    flags_d = nc.dram_tensor("flags", [128, NT], F32, kind="ExternalInput").ap()
    bA_d = nc.dram_tensor("bA", [128, 1], F32, kind="ExternalInput").ap()
    Wn_d = nc.dram_tensor("Wn", [f, f], F32, kind="ExternalInput").ap()
    bWn_d = nc.dram_tensor("bWn", [f, 1], F32, kind="ExternalInput").ap()
    As_d = nc.dram_tensor("As", [f, 2], F32, kind="ExternalInput").ap()
    whnT = nc.dram_tensor("whnT", [f, NT * 128], F32, kind="ExternalOutput").ap()
    sn = nc.dram_tensor("sn", [2, NT * 128], F32, kind="ExternalOutput").ap()

    X = mybir.AxisListType.X
    AF = mybir.ActivationFunctionType
    OP = mybir.AluOpType

    def v(ap, dims, off=0):
        return dataclasses.replace(
            ap,
            ap=[list(ap.ap[0])] + [list(d) for d in dims],
            offset=ap.offset + off,
        )

    nq = min(4, nc.num_swdge_queues)

    with tile.TileContext(nc) as tc:
        with tc.tile_pool(name="const", bufs=1) as pc, tc.tile_pool(
            name="io", bufs=3
        ) as pio, tc.tile_pool(name="work", bufs=2) as pw, tc.tile_pool(
            name="ps", bufs=2, space="PSUM"
        ) as pps, tc.tile_pool(name="ep", bufs=2) as pep:
            si_sb = pc.tile([128, NT], F32)
            nc.sync.dma_start(out=si_sb[:], in_=si_d[:, :])
            flags_sb = pc.tile([128, NT], F32)
            nc.sync.dma_start(out=flags_sb[:], in_=flags_d[:, :])
            em_sb = pc.tile([128, WTOT], F32)
            nc.sync.dma_start(out=em_sb[:], in_=em_d[:, :])
            bA_sb = pc.tile([128, 1], F32)
            nc.sync.dma_start(out=bA_sb[:], in_=bA_d[:, :])
            Wn_sb = pc.tile([f, f], F32)
            nc.sync.dma_start(out=Wn_sb[:], in_=Wn_d[:, :])
            bWn_sb = pc.tile([f, 1], F32)
            nc.sync.dma_start(out=bWn_sb[:], in_=bWn_d[:, :])
            As_sb = pc.tile([f, 2], F32)
            nc.sync.dma_start(out=As_sb[:], in_=As_d[:, :])
            ident = pc.tile([128, 128], F32)
            make_identity(nc, ident[:])

            hTL = None
            CHT = 4  # tiles per epilogue chunk (512 dsts; matmul N<=512 fp32)

            def flush_chunk(ck, ntl):
                cols = ntl * 128
                ps_w = pps.tile([f, CHT * 128], F32, tag="psw", space="PSUM")
                nc.tensor.matmul(
                    out=ps_w[:, :cols],
                    lhsT=Wn_sb[:],
                    rhs=hTL[:, :cols],
                    start=True,
                    stop=True,
                )
                whn_sb = pep.tile([f, CHT * 128], F32, tag="whn")
                nc.scalar.activation(
                    out=whn_sb[:, :cols],
                    in_=ps_w[:, :cols],
                    func=AF.Identity,
                    bias=bWn_sb[:],
                )
                nc.sync.dma_start(
                    out=whnT[:, ck * CHT * 128 : ck * CHT * 128 + cols],
                    in_=whn_sb[:, :cols],
                )
                ps_s = pps.tile([2, CHT * 128], F32, tag="pss", space="PSUM")
                nc.tensor.matmul(
                    out=ps_s[:, :cols],
                    lhsT=As_sb[:],
                    rhs=whn_sb[:, :cols],
                    start=True,
                    stop=True,
                )
                s_sb = pep.tile([2, CHT * 128], F32, tag="ssb")
                nc.vector.tensor_copy(out=s_sb[:, :cols], in_=ps_s[:, :cols])
                nc.sync.dma_start(
                    out=sn[:, ck * CHT * 128 : ck * CHT * 128 + cols],
                    in_=s_sb[:, :cols],
                )

            for ci, (s, t0, ntc, D, col0) in enumerate(sched.calls):
                W = ntc * D
                n_idx = 128 * W
                idx_sb = pio.tile([128, 8 * W], I16, tag="idx")
                nc.sync.dma_start(
                    out=idx_sb[:], in_=idx_d[:, 8 * col0 : 8 * (col0 + W)]
                )
                gbuf = pw.tile([128, W * f], F32, tag="gbuf")
                # hw limit: <=1024 descriptors per dma_gather -> <=8 columns
                for j0 in range(0, W, 8):
                    jw = min(8, W - j0)
                    nc.gpsimd.dma_gather(
                        out_ap=v(gbuf[:], [(f, jw), (1, f)], off=j0 * f),
                        in_ap=table[s * RSUB : (s + 1) * RSUB, :],
                        idxs_ap=idx_sb[:, 8 * j0 : 8 * (j0 + jw)],
                        num_idxs=128 * jw,
                        num_idxs_reg=128 * jw,
                        elem_size=f,
                        queue_num=ci % nq,
                    )
                # e_pre = sj + si ; sj at f32 slot 32 of each 64-f32 row
                epre = pw.tile([128, W], F32, tag="epre")
                sj_view = v(gbuf[:], [(f * D, ntc), (f, D)], off=32)
                si_bc = si_sb[:, t0 : t0 + ntc].to_broadcast([128, ntc, D])
                nc.vector.tensor_tensor(
                    out=v(epre[:], [(D, ntc), (1, D)]),
                    in0=sj_view,
                    in1=si_bc,
                    op=OP.add,
                )
                # leaky(e_pre + bA), then add pad mask
                e1 = pw.tile([128, W], F32, tag="e1")
                nc.scalar.activation(
                    out=e1[:], in_=epre[:], func=AF.Identity, bias=bA_sb[:]
                )
                e2 = pw.tile([128, W], F32, tag="e2")
                nc.vector.tensor_scalar(
                    out=e2[:], in0=e1[:], scalar1=ALPHA, scalar2=None, op0=OP.mult
                )
                nc.vector.tensor_tensor(out=e1[:], in0=e1[:], in1=e2[:], op=OP.max)
                nc.vector.tensor_tensor(
                    out=e1[:], in0=e1[:], in1=em_sb[:, col0 : col0 + W], op=OP.add
                )
                # segment softmax over slot axis
                m = pw.tile([128, ntc], F32, tag="m")
                nc.vector.tensor_reduce(
                    out=m[:], in_=v(e1[:], [(D, ntc), (1, D)]), axis=X, op=OP.max
                )
                nc.vector.tensor_tensor(
                    out=v(e1[:], [(D, ntc), (1, D)]),
                    in0=v(e1[:], [(D, ntc), (1, D)]),
                    in1=m[:].to_broadcast([128, ntc, D]),
                    op=OP.subtract,
                )
                ex = pw.tile([128, W], F32, tag="ex")
                nc.scalar.activation(out=ex[:], in_=e1[:], func=AF.Exp)
                den = pw.tile([128, ntc], F32, tag="den")
                nc.vector.tensor_reduce(
                    out=den[:], in_=v(ex[:], [(D, ntc), (1, D)]), axis=X, op=OP.add
                )
                rden = pw.tile([128, ntc], F32, tag="rden")
                nc.vector.tensor_scalar(
                    out=den[:], in0=den[:], scalar1=EPS, scalar2=None, op0=OP.add
                )
                nc.vector.reciprocal(out=rden[:], in_=den[:])
                nc.vector.tensor_tensor(
                    out=rden[:], in0=rden[:], in1=flags_sb[:, t0 : t0 + ntc], op=OP.mult
                )
                # weighted message sum (Wh is fp16-packed in first 128B of row)
                gbuf16 = gbuf[:].bitcast(F16)
                msgw = pw.tile([128, W * f], F32, tag="msgw")
                nc.vector.tensor_tensor(
                    out=v(msgw[:], [(f, W), (1, f)]),
                    in0=v(gbuf16, [(2 * f, W), (1, f)]),
                    in1=ex[:].to_broadcast([128, W, f]),
                    op=OP.mult,
                )
                hc = pw.tile([128, ntc * f], F32, tag="hc")
                nc.vector.tensor_reduce(
                    out=hc[:],
                    in_=v(msgw[:], [(D * f, ntc), (1, f), (f, D)]),
                    axis=X,
                    op=OP.add,
                )
                for tl in range(ntc):
                    t = t0 + tl
                    ht = pw.tile([128, f], F32, tag="ht")
                    nc.vector.tensor_scalar(
                        out=ht[:],
                        in0=hc[:, tl * f : (tl + 1) * f],
                        scalar1=rden[:, tl : tl + 1],
                        scalar2=None,
                        op0=OP.mult,
                    )
                    ps_t = pps.tile([f, 128], F32, tag="pst", space="PSUM")
                    nc.tensor.transpose(out=ps_t[:], in_=ht[:], identity=ident[:])
                    if t % CHT == 0:
                        hTL = pep.tile([f, CHT * 128], F32, tag="hTL")
                    j = t % CHT
                    nc.scalar.activation(
                        out=hTL[:, j * 128 : (j + 1) * 128],
                        in_=ps_t[:],
                        func=AF.Identity,
                        scale=ALPHA,
                    )
                    e3 = pep.tile([f, 128], F32, tag="e3")
                    nc.vector.tensor_copy(out=e3[:], in_=ps_t[:])
                    nc.vector.tensor_tensor(
                        out=hTL[:, j * 128 : (j + 1) * 128],
                        in0=hTL[:, j * 128 : (j + 1) * 128],
                        in1=e3[:],
                        op=OP.max,
                    )
                    if t % CHT == CHT - 1 or t == NT - 1:
                        flush_chunk(t // CHT, t % CHT + 1)
    nc.compile()
    return nc


# ---------------------------------------------------------------- driver
_cache = {}


def _pack_table(sched, c, wh_full, sj_full):
    table = np.zeros((NSUB * RSUB, F), np.float32)
    for s in range(NSUB):
        u = sched.uniq[c][s]
        if not len(u):
            continue
        blk16 = wh_full[u].astype(np.float16)  # [ns, 64]
        table[s * RSUB : s * RSUB + len(u), :32] = blk16.view(np.float32)
        table[s * RSUB : s * RSUB + len(u), 32] = sj_full[u]
    return table


def kernel(x, edge_index, W1, bW1, A1, bA1, W2, bW2, A2, bA2, Wfc, bfc):
    x = np.asarray(x, dtype=np.float32)
    edge_index = np.asarray(edge_index)
    W1 = np.asarray(W1, np.float32)
    bW1 = np.asarray(bW1, np.float32)
    A1 = np.asarray(A1, np.float32)
    bA1 = np.asarray(bA1, np.float32)
    W2 = np.asarray(W2, np.float32)
    bW2 = np.asarray(bW2, np.float32)
    A2 = np.asarray(A2, np.float32)
    bA2 = np.asarray(bA2, np.float32)
    Wfc = np.asarray(Wfc, np.float32)
    bfc = np.asarray(bfc, np.float32)

    sched = build_schedule(edge_index)
    cores = list(range(N_CORES))

    if "A" not in _cache:
        _cache["A"] = build_progA()
    ncA = _cache["A"]
    inA = []
    for c in cores:
        xT = np.ascontiguousarray(x[c * DPC : (c + 1) * DPC].T)
        inA.append(
            {
                "xT": xT,
                "W": W1,
                "bW": bW1.reshape(F, 1),
                "As": np.ascontiguousarray(np.concatenate([A1[:F], A1[F:]], axis=1)),
            }
        )
    resA = bass_utils.run_bass_kernel_spmd(ncA, inA, core_ids=cores)
    wh = np.concatenate([resA.results[c]["whT"].T for c in cores], axis=0)
    s_all = np.concatenate([resA.results[c]["s"] for c in cores], axis=1)
    si_full, sj_full = s_all[0], s_all[1]

    key = ("B", sched.n_tiles, sched.w_total, tuple(sched.tiles))
    if key not in _cache:
        _cache[key] = build_progB(sched)
    ncB = _cache[key]

    def launch_B(wh_full, si_f, sj_f, bA, Wn, bWn, An):
        inB = []
        for c in cores:
            perm = sched.perms[c]
            real = perm >= 0
            gids = c * DPC + perm[real]
            tmp = np.zeros(sched.n_tiles * 128, np.float32)
            tmp[real] = si_f[gids]
            si_arr = np.ascontiguousarray(tmp.reshape(sched.n_tiles, 128).T)
            inB.append(
                {
                    "table": _pack_table(sched, c, wh_full, sj_f),
                    "idx": sched.idx16[c],
                    "emask": sched.emask[c],
                    "si": si_arr,
                    "flags": sched.flags[c],
                    "bA": np.full((128, 1), bA.reshape(-1)[0], np.float32),
                    "Wn": Wn,
                    "bWn": bWn.reshape(F, 1),
                    "As": An,
                }
            )
        res = bass_utils.run_bass_kernel_spmd(ncB, inB, core_ids=cores)
        whn = np.zeros((N_NODES, F), np.float32)
        sn_i = np.zeros(N_NODES, np.float32)
        sn_j = np.zeros(N_NODES, np.float32)
        for c in cores:
            perm = sched.perms[c]
            real = perm >= 0
            gids = c * DPC + perm[real]
            whn[gids] = res.results[c]["whnT"].T[real]
            sn_c = res.results[c]["sn"]
            sn_i[gids] = sn_c[0][real]
            sn_j[gids] = sn_c[1][real]
        return whn, sn_i, sn_j

    As2 = np.ascontiguousarray(np.concatenate([A2[:F], A2[F:]], axis=1))
    wh2, si2, sj2 = launch_B(wh, si_full, sj_full, bA1, W2, bW2, As2)
    out, _, _ = launch_B(wh2, si2, sj2, bA2, Wfc, bfc, np.zeros((F, 2), np.float32))
    return out.astype(np.float32)



# revision 9
# speedup vs baseline: 7.0238x; 7.0238x over previous
"""GAT (2-layer) on 8 NeuronCores — Bass/Tile kernel.

Strategy (dst-sharded graph parallel, host-expanded dense streams):
  - Each core owns 12500 destination nodes, degree-sorted into 128-dst
    tiles; tiles are paired and grouped into calls with a shared
    per-call slot capacity D (cross-core max), giving a dense
    [128 dst x D slot] layout per tile.
  - Launch A: per-core Wh1^T = (x W1 + b)^T and attention scalars
    s_i/s_j (all model FLOPs on device).
  - Host pre/re-pack (pure indexing of device-computed values): expands
    the per-edge source stream  stream16[p, col, f] = Wh[src] (fp16),
    sj_slot[p, col] = s_j[src] (f32, -1e30 at pad slots), si per tile,
    zero-degree flags.  No arithmetic on features happens on host.
  - Launch B (x2, one per GAT layer): streams the dense fp16 tables at
    line rate (plain dma_start, no gathers), computes masked segment
    softmax over the slot axis, alpha-weighted message sum (fp16
    multiply in place, f32 accumulate), leaky-relu, and the epilogue
    matmul with the next layer's weights (block-diagonal pair trick)
    -> next-layer Wh^T + attention scalars (or final fc output).
"""

import dataclasses
import numpy as np

import concourse.bacc as bacc
import concourse.tile as tile
from concourse import bass, mybir, bass_utils
from concourse.masks import make_identity

F32 = mybir.dt.float32
F16 = mybir.dt.float16

N_NODES = 100000
N_CORES = 8
DPC = N_NODES // N_CORES
F = 64
IN_C = 128
NEG_BIG = -1.0e30
ALPHA = 0.2
CALL_W = 256  # max slot-columns per call chunk
FLUSH_PAIRS = 4  # tile-pairs per epilogue matmul (512 psum cols)


@dataclasses.dataclass
class Schedule:
    n_tiles: int  # tiles per core (even)
    w_total: int  # total slot columns
    calls: list  # (t0, ntc, D, col0) ; ntc even
    gids: np.ndarray  # [N_CORES, n_tiles*128] global dst id or -1
    slot_src: np.ndarray  # [N_CORES, 128, w_total] src id or N_NODES (pad)
    si_gid: np.ndarray  # [N_CORES, 128, n_tiles] dst gid clipped (for si gather)
    flags: np.ndarray  # [N_CORES, 128, n_tiles] f32 1.0 where real dst with deg>0
    tile_col0: np.ndarray  # [n_tiles] starting col of each tile
    tile_D: np.ndarray  # [n_tiles] capacity of each tile


def build_schedule(edge_index: np.ndarray) -> Schedule:
    src = np.asarray(edge_index[0], dtype=np.int64)
    dst = np.asarray(edge_index[1], dtype=np.int64)
    order = np.argsort(dst, kind="stable")
    src_s = src[order]
    deg_all = np.bincount(dst, minlength=N_NODES).astype(np.int64)
    starts_all = np.concatenate([[0], np.cumsum(deg_all)])

    n_tiles = -(-DPC // 128)
    if n_tiles % 2:
        n_tiles += 1
    ntile_slots = n_tiles * 128

    # per-core degree-sorted dst order, padded with -1
    gids = np.full((N_CORES, ntile_slots), -1, np.int64)
    for c in range(N_CORES):
        degc = deg_all[c * DPC : (c + 1) * DPC]
        rank = np.argsort(degc, kind="stable")
        gids[c, :DPC] = c * DPC + rank

    deg_pad = np.concatenate([deg_all, [0]])
    gclip = np.where(gids >= 0, gids, N_NODES)
    degs = deg_pad[gclip].reshape(N_CORES, n_tiles, 128)
    tile_max = degs.max(axis=2).max(axis=0)  # [n_tiles] cross-core max deg

    # call plan over tile PAIRS: group pairs while ntc*D <= CALL_W
    pair_max = np.maximum(tile_max[0::2], tile_max[1::2])
    calls = []
    col = 0
    p0 = 0
    n_pairs = n_tiles // 2
    while p0 < n_pairs:
        D = max(1, int(pair_max[p0]))
        npair = 1
        while p0 + npair < n_pairs:
            nd = max(D, int(pair_max[p0 + npair]))
            if (npair + 1) * 2 * nd > CALL_W:
                break
            D = nd
            npair += 1
        calls.append((2 * p0, 2 * npair, D, col))
        col += 2 * npair * D
        p0 += npair
    w_total = col

    tile_col0 = np.zeros(n_tiles, np.int64)
    tile_D = np.zeros(n_tiles, np.int64)
    for (t0, ntc, D, col0) in calls:
        for tl in range(ntc):
            tile_col0[t0 + tl] = col0 + tl * D
            tile_D[t0 + tl] = D

    # slot_src: vectorized CSR -> padded-slot scatter
    slot_src = np.full((N_CORES, 128, w_total), N_NODES, np.int64)
    colstart_of_slot = tile_col0[
        np.arange(ntile_slots) // 128
    ]  # [ntile_slots] per (tile,partition)
    for c in range(N_CORES):
        g = gclip[c]
        ne = deg_pad[g]
        p_of_slot = np.arange(ntile_slots) % 128
        # flat positions in [128, w_total]: p*w_total + colstart + d
        base = p_of_slot * w_total + colstart_of_slot
        tot = int(ne.sum())
        pos = np.repeat(base, ne) + (
            np.arange(tot) - np.repeat(np.cumsum(ne) - ne, ne)
        )
        srcidx = np.repeat(starts_all[g], ne) + (
            np.arange(tot) - np.repeat(np.cumsum(ne) - ne, ne)
        )
        flat = slot_src[c].reshape(-1)
        flat[pos] = src_s[srcidx]

    si_gid = gclip.reshape(N_CORES, n_tiles, 128).transpose(0, 2, 1)
    flags = (
        ((gids >= 0) & (deg_pad[gclip] > 0))
        .reshape(N_CORES, n_tiles, 128)
        .transpose(0, 2, 1)
        .astype(np.float32)
    )
    flags = np.ascontiguousarray(flags)
    si_gid = np.ascontiguousarray(si_gid)

    return Schedule(
        n_tiles, w_total, calls, gids, slot_src, si_gid, flags, tile_col0, tile_D
    )


# ---------------------------------------------------------------- prog A
def build_progA(n_loc=DPC, in_c=IN_C, f=F):
    nc = bacc.Bacc("TRN2", target_bir_lowering=False, debug=False, num_devices=N_CORES)
    xT = nc.dram_tensor("xT", [in_c, n_loc], F32, kind="ExternalInput").ap()
    W = nc.dram_tensor("W", [in_c, f], F32, kind="ExternalInput").ap()
    bW = nc.dram_tensor("bW", [f, 1], F32, kind="ExternalInput").ap()
    As = nc.dram_tensor("As", [f, 2], F32, kind="ExternalInput").ap()
    whT = nc.dram_tensor("whT", [f, n_loc], F32, kind="ExternalOutput").ap()
    s = nc.dram_tensor("s", [2, n_loc], F32, kind="ExternalOutput").ap()

    with tile.TileContext(nc) as tc:
        with tc.tile_pool(name="sb", bufs=1) as pool, tc.tile_pool(
            name="ps", bufs=2, space="PSUM"
        ) as pps, tc.tile_pool(name="sb2", bufs=3) as pool2:
            xT_sb = pool.tile([in_c, n_loc], F32)
            nc.sync.dma_start(out=xT_sb[:], in_=xT[:, :])
            W_sb = pool.tile([in_c, f], F32)
            nc.sync.dma_start(out=W_sb[:], in_=W[:, :])
            bW_sb = pool.tile([f, 1], F32)
            nc.sync.dma_start(out=bW_sb[:], in_=bW[:, :])
            As_sb = pool.tile([f, 2], F32)
            nc.sync.dma_start(out=As_sb[:], in_=As[:, :])

            CH = 512
            for c0 in range(0, n_loc, CH):
                ch = min(CH, n_loc - c0)
                ps_w = pps.tile([f, CH], F32, space="PSUM")
                nc.tensor.matmul(
                    out=ps_w[:, :ch],
                    lhsT=W_sb[:],
                    rhs=xT_sb[:, c0 : c0 + ch],
                    start=True,
                    stop=True,
                )
                wh_sb = pool2.tile([f, CH], F32, tag="wh")
                nc.scalar.activation(
                    out=wh_sb[:, :ch],
                    in_=ps_w[:, :ch],
                    func=mybir.ActivationFunctionType.Identity,
                    bias=bW_sb[:],
                )
                nc.sync.dma_start(out=whT[:, c0 : c0 + ch], in_=wh_sb[:, :ch])
                ps_s = pps.tile([2, CH], F32, space="PSUM")
                nc.tensor.matmul(
                    out=ps_s[:, :ch],
                    lhsT=As_sb[:],
                    rhs=wh_sb[:, :ch],
                    start=True,
                    stop=True,
                )
                s_sb = pool2.tile([2, CH], F32, tag="ssb")
                nc.vector.tensor_copy(out=s_sb[:, :ch], in_=ps_s[:, :ch])
                nc.sync.dma_start(out=s[:, c0 : c0 + ch], in_=s_sb[:, :ch])
    nc.compile()
    return nc


# ---------------------------------------------------------------- prog B
def build_progB(sched: Schedule, f=F):
    NT = sched.n_tiles
    WTOT = sched.w_total
    NPAIR = NT // 2
    nc = bacc.Bacc("TRN2", target_bir_lowering=False, debug=False, num_devices=N_CORES)
    stream = nc.dram_tensor("stream", [128, WTOT * f], F16, kind="ExternalInput").ap()
    sj_d = nc.dram_tensor("sj", [128, WTOT], F32, kind="ExternalInput").ap()
    si_d = nc.dram_tensor("si", [128, NT], F32, kind="ExternalInput").ap()
    flags_d = nc.dram_tensor("flags", [128, NT], F32, kind="ExternalInput").ap()
    bA_d = nc.dram_tensor("bA", [128, 1], F32, kind="ExternalInput").ap()
    WnBD_d = nc.dram_tensor("WnBD", [128, 128], F32, kind="ExternalInput").ap()
    bWn_d = nc.dram_tensor("bWn", [128, 1], F32, kind="ExternalInput").ap()
    AsBD_d = nc.dram_tensor("AsBD", [128, 4], F32, kind="ExternalInput").ap()
    whnT = nc.dram_tensor("whnT", [128, NPAIR * 128], F32, kind="ExternalOutput").ap()
    sn = nc.dram_tensor("sn", [4, NPAIR * 128], F32, kind="ExternalOutput").ap()

    X = mybir.AxisListType.X
    AF = mybir.ActivationFunctionType
    OP = mybir.AluOpType
    MAXNTC = max(ntc for (_, ntc, _, _) in sched.calls)

    def v(ap, dims, off=0):
        return dataclasses.replace(
            ap,
            ap=[list(ap.ap[0])] + [list(d) for d in dims],
            offset=ap.offset + off,
        )

    with tile.TileContext(nc) as tc:
        with tc.tile_pool(name="const", bufs=1) as pc, tc.tile_pool(
            name="io", bufs=3
        ) as pio, tc.tile_pool(name="work", bufs=2) as pw, tc.tile_pool(
            name="ps", bufs=2, space="PSUM"
        ) as pps, tc.tile_pool(name="ps2", bufs=2, space="PSUM") as pps2, tc.tile_pool(
            name="ep", bufs=2
        ) as pep:
            sj_sb = pc.tile([128, WTOT], F32)
            nc.sync.dma_start(out=sj_sb[:], in_=sj_d[:, :])
            si_sb = pc.tile([128, NT], F32)
            nc.sync.dma_start(out=si_sb[:], in_=si_d[:, :])
            flags_sb = pc.tile([128, NT], F32)
            nc.sync.dma_start(out=flags_sb[:], in_=flags_d[:, :])
            bA_sb = pc.tile([128, 1], F32)
            nc.sync.dma_start(out=bA_sb[:], in_=bA_d[:, :])
            WnBD_sb = pc.tile([128, 128], F32)
            nc.sync.dma_start(out=WnBD_sb[:], in_=WnBD_d[:, :])
            bWn_sb = pc.tile([128, 1], F32)
            nc.sync.dma_start(out=bWn_sb[:], in_=bWn_d[:, :])
            AsBD_sb = pc.tile([128, 4], F32)
            nc.sync.dma_start(out=AsBD_sb[:], in_=AsBD_d[:, :])
            ident = pc.tile([128, 128], F32)
            make_identity(nc, ident[:])

            # epilogue flush state: stacked-pair h columns awaiting matmul
            state = {"hgrp": None, "k0": 0, "n": 0}

            def flush_pairs():
                if not state["n"]:
                    return
                hgrp = state["hgrp"]
                k0 = state["k0"]
                cols = state["n"] * 128
                ps_w = pps2.tile([128, FLUSH_PAIRS * 128], F32, tag="psw", space="PSUM")
                nc.tensor.matmul(
                    out=ps_w[:, :cols],
                    lhsT=WnBD_sb[:],
                    rhs=hgrp[:, :cols],
                    start=True,
                    stop=True,
                )
                whn_sb = pep.tile([128, FLUSH_PAIRS * 128], F32, tag="whn")
                nc.scalar.activation(
                    out=whn_sb[:, :cols],
                    in_=ps_w[:, :cols],
                    func=AF.Identity,
                    bias=bWn_sb[:],
                )
                nc.sync.dma_start(
                    out=whnT[:, k0 * 128 : k0 * 128 + cols], in_=whn_sb[:, :cols]
                )
                ps_s = pps2.tile([4, FLUSH_PAIRS * 128], F32, tag="pss", space="PSUM")
                nc.tensor.matmul(
                    out=ps_s[:, :cols],
                    lhsT=AsBD_sb[:],
                    rhs=whn_sb[:, :cols],
                    start=True,
                    stop=True,
                )
                s_sb = pep.tile([4, FLUSH_PAIRS * 128], F32, tag="ssb")
                nc.vector.tensor_copy(out=s_sb[:, :cols], in_=ps_s[:, :cols])
                nc.sync.dma_start(
                    out=sn[:, k0 * 128 : k0 * 128 + cols], in_=s_sb[:, :cols]
                )
                state["hgrp"] = None
                state["n"] = 0

            for (t0, ntc, D, col0) in sched.calls:
                W = ntc * D
                st = pio.tile([128, CALL_W * f], F16, tag="st")
                nc.sync.dma_start(
                    out=st[:, : W * f], in_=stream[:, col0 * f : (col0 + W) * f]
                )
                # e = leaky(s_j + s_i + bA); pads carry -1e30 inside sj
                epre = pw.tile([128, CALL_W], F32, tag="epre")
                nc.vector.tensor_tensor(
                    out=v(epre[:], [(D, ntc), (1, D)]),
                    in0=v(sj_sb[:], [(D, ntc), (1, D)], off=col0),
                    in1=si_sb[:, t0 : t0 + ntc].to_broadcast([128, ntc, D]),
                    op=OP.add,
                )
                e1 = pw.tile([128, CALL_W], F32, tag="e1")
                nc.scalar.activation(
                    out=e1[:, :W],
                    in_=epre[:, :W],
                    func=AF.Prelu,
                    bias=bA_sb[:],
                    alpha=ALPHA,
                )
                # segment softmax over slot axis
                m = pw.tile([128, MAXNTC], F32, tag="m")
                nc.vector.tensor_reduce(
                    out=m[:, :ntc],
                    in_=v(e1[:], [(D, ntc), (1, D)]),
                    axis=X,
                    op=OP.max,
                )
                nc.vector.tensor_tensor(
                    out=v(e1[:], [(D, ntc), (1, D)]),
                    in0=v(e1[:], [(D, ntc), (1, D)]),
                    in1=m[:, :ntc].to_broadcast([128, ntc, D]),
                    op=OP.subtract,
                )
                ex = pw.tile([128, CALL_W], F32, tag="ex")
                nc.scalar.activation(out=ex[:, :W], in_=e1[:, :W], func=AF.Exp)
                den = pw.tile([128, MAXNTC], F32, tag="den")
                nc.vector.tensor_reduce(
                    out=den[:, :ntc],
                    in_=v(ex[:], [(D, ntc), (1, D)]),
                    axis=X,
                    op=OP.add,
                )
                rnorm = pw.tile([128, MAXNTC], F32, tag="rnorm")
                nc.vector.reciprocal(out=rnorm[:, :ntc], in_=den[:, :ntc])
                nc.vector.tensor_tensor(
                    out=rnorm[:, :ntc],
                    in0=rnorm[:, :ntc],
                    in1=flags_sb[:, t0 : t0 + ntc],
                    op=OP.mult,
                )
                exn = pw.tile([128, CALL_W], F16, tag="exn")
                nc.vector.tensor_tensor(
                    out=v(exn[:], [(D, ntc), (1, D)]),
                    in0=v(ex[:], [(D, ntc), (1, D)]),
                    in1=rnorm[:, :ntc].to_broadcast([128, ntc, D]),
                    op=OP.mult,
                )
                # weighted messages in place over the stream tile (fp16)
                nc.vector.tensor_tensor(
                    out=v(st[:], [(f, W), (1, f)]),
                    in0=v(st[:], [(f, W), (1, f)]),
                    in1=exn[:, :W].to_broadcast([128, W, f]),
                    op=OP.mult,
                )
                hc = pw.tile([128, MAXNTC * f], F32, tag="hc")
                nc.vector.tensor_reduce(
                    out=hc[:, : ntc * f],
                    in_=v(st[:], [(D * f, ntc), (1, f), (f, D)]),
                    axis=X,
                    op=OP.add,
                )
                # epilogue per tile pair: transpose + leaky into the flush group
                for pr in range(ntc // 2):
                    kpair = (t0 + 2 * pr) // 2
                    ps_t = pps.tile([128, 128], F32, tag="pst", space="PSUM")
                    nc.tensor.transpose(
                        out=ps_t[:],
                        in_=hc[:, 2 * pr * f : (2 * pr + 2) * f],
                        identity=ident[:],
                    )
                    if state["n"] == 0:
                        state["hgrp"] = pep.tile(
                            [128, FLUSH_PAIRS * 128], F32, tag="hgrp", name="hgrp"
                        )
                        state["k0"] = kpair
                    j = state["n"]
                    nc.scalar.activation(
                        out=state["hgrp"][:, j * 128 : (j + 1) * 128],
                        in_=ps_t[:],
                        func=AF.Prelu,
                        alpha=ALPHA,
                    )
                    state["n"] += 1
                    if state["n"] == FLUSH_PAIRS:
                        flush_pairs()
            flush_pairs()
    nc.compile()
    return nc


# ---------------------------------------------------------------- driver
_cache = {}


def kernel(x, edge_index, W1, bW1, A1, bA1, W2, bW2, A2, bA2, Wfc, bfc):
    x = np.asarray(x, dtype=np.float32)
    edge_index = np.asarray(edge_index)
    W1 = np.asarray(W1, np.float32)
    bW1 = np.asarray(bW1, np.float32)
    A1 = np.asarray(A1, np.float32)
    bA1 = np.asarray(bA1, np.float32)
    W2 = np.asarray(W2, np.float32)
    bW2 = np.asarray(bW2, np.float32)
    A2 = np.asarray(A2, np.float32)
    bA2 = np.asarray(bA2, np.float32)
    Wfc = np.asarray(Wfc, np.float32)
    bfc = np.asarray(bfc, np.float32)

    sched = build_schedule(edge_index)
    cores = list(range(N_CORES))
    NT = sched.n_tiles
    NPAIR = NT // 2

    if "A" not in _cache:
        _cache["A"] = build_progA()
    ncA = _cache["A"]
    inA = []
    for c in cores:
        xT = np.ascontiguousarray(x[c * DPC : (c + 1) * DPC].T)
        inA.append(
            {
                "xT": xT,
                "W": W1,
                "bW": bW1.reshape(F, 1),
                "As": np.ascontiguousarray(np.concatenate([A1[:F], A1[F:]], axis=1)),
            }
        )
    resA = bass_utils.run_bass_kernel_spmd(ncA, inA, core_ids=cores)
    wh = np.concatenate([resA.results[c]["whT"].T for c in cores], axis=0)
    s_all = np.concatenate([resA.results[c]["s"] for c in cores], axis=1)
    si_full, sj_full = s_all[0], s_all[1]

    key = ("B", NT, sched.w_total, tuple(sched.calls))
    if key not in _cache:
        _cache[key] = build_progB(sched)
    ncB = _cache[key]

    def launch_B(wh_full, si_f, sj_f, bA, Wn, bWn, An):
        wh16 = np.concatenate(
            [wh_full.astype(np.float16), np.zeros((1, F), np.float16)], axis=0
        )
        sjpad = np.concatenate([sj_f, [np.float32(NEG_BIG)]]).astype(np.float32)
        sipad = np.concatenate([si_f, [np.float32(0.0)]]).astype(np.float32)
        WnBD = np.zeros((128, 128), np.float32)
        WnBD[:F, :F] = Wn
        WnBD[F:, F:] = Wn
        AsBD = np.zeros((128, 4), np.float32)
        AsBD[:F, 0:1] = An[:, 0:1]
        AsBD[:F, 1:2] = An[:, 1:2]
        AsBD[F:, 2:3] = An[:, 0:1]
        AsBD[F:, 3:4] = An[:, 1:2]
        bWn2 = np.concatenate([bWn.reshape(F), bWn.reshape(F)]).reshape(128, 1)
        inB = []
        for c in cores:
            ss = sched.slot_src[c]
            inB.append(
                {
                    "stream": wh16[ss].reshape(128, sched.w_total * F),
                    "sj": sjpad[ss],
                    "si": sipad[sched.si_gid[c]],
                    "flags": sched.flags[c],
                    "bA": np.full((128, 1), bA.reshape(-1)[0], np.float32),
                    "WnBD": WnBD,
                    "bWn": bWn2,
                    "AsBD": AsBD,
                }
            )
        res = bass_utils.run_bass_kernel_spmd(ncB, inB, core_ids=cores)
        whn = np.zeros((N_NODES, F), np.float32)
        sn_i = np.zeros(N_NODES, np.float32)
        sn_j = np.zeros(N_NODES, np.float32)
        for c in cores:
            gids = sched.gids[c]
            real = gids >= 0
            w = res.results[c]["whnT"].reshape(128, NPAIR, 128)
            snc = res.results[c]["sn"].reshape(4, NPAIR, 128)
            # tile 2k -> rows 0:64 of pair k; tile 2k+1 -> rows 64:128
            wA = w[:F].transpose(1, 2, 0)  # [NPAIR, 128, F] even tiles
            wB = w[F:].transpose(1, 2, 0)  # odd tiles
            wfull = np.empty((NT, 128, F), np.float32)
            wfull[0::2] = wA
            wfull[1::2] = wB
            sfull_i = np.empty((NT, 128), np.float32)
            sfull_j = np.empty((NT, 128), np.float32)
            sfull_i[0::2] = snc[0]
            sfull_i[1::2] = snc[2]
            sfull_j[0::2] = snc[1]
            sfull_j[1::2] = snc[3]
            whn[gids[real]] = wfull.reshape(NT * 128, F)[real]
            sn_i[gids[real]] = sfull_i.reshape(-1)[real]
            sn_j[gids[real]] = sfull_j.reshape(-1)[real]
        return whn, sn_i, sn_j

    As2 = np.ascontiguousarray(np.concatenate([A2[:F], A2[F:]], axis=1))
    wh2, si2, sj2 = launch_B(wh, si_full, sj_full, bA1, W2, bW2, As2)
    out, _, _ = launch_B(wh2, si2, sj2, bA2, Wfc, bfc, np.zeros((F, 2), np.float32))
    return out.astype(np.float32)


# revision 16
# speedup vs baseline: 10.7101x; 1.5248x over previous
"""GAT (2-layer) on 8 NeuronCores — Bass/Tile kernel.

Strategy (dst-sharded graph parallel, host-expanded dense streams):
  - Each core owns 12500 destination nodes, degree-sorted into 128-dst
    tiles; tiles are paired and grouped into calls with a shared
    per-call slot capacity D (cross-core max), giving a dense
    [128 dst x D slot] layout per tile.
  - Launch A: per-core Wh1^T = (x W1 + b)^T and attention scalars
    s_i/s_j (all model FLOPs on device).
  - Host pre/re-pack (pure indexing of device-computed values): expands
    the per-edge source stream  stream16[p, col, f] = Wh[src] (fp16),
    sj_slot[p, col] = s_j[src] (f32, -1e30 at pad slots), si per tile,
    zero-degree flags.  No arithmetic on features happens on host.
  - Launch B (x2, one per GAT layer): streams the dense fp16 tables at
    line rate (plain dma_start, no gathers), computes masked segment
    softmax over the slot axis, alpha-weighted message sum (fp16
    multiply in place, f32 accumulate), leaky-relu, and the epilogue
    matmul with the next layer's weights (block-diagonal pair trick)
    -> next-layer Wh^T + attention scalars (or final fc output).
"""

import dataclasses
import numpy as np

import concourse.bacc as bacc
import concourse.tile as tile
from concourse import bass, mybir, bass_utils
from concourse.masks import make_identity

F32 = mybir.dt.float32
F16 = mybir.dt.float16

N_NODES = 100000
N_CORES = 8
DPC = N_NODES // N_CORES
F = 64
IN_C = 128
NEG_BIG = -1.0e30
ALPHA = 0.2
CALL_W = 256  # max slot-columns per call chunk
FLUSH_PAIRS = 4  # tile-pairs per epilogue matmul (512 psum cols)


@dataclasses.dataclass
class Schedule:
    n_tiles: int  # tiles per core (even)
    w_total: int  # total slot columns
    calls: list  # (t0, ntc, D, col0) ; ntc even
    gids: np.ndarray  # [N_CORES, n_tiles*128] global dst id or -1
    slot_src: np.ndarray  # [N_CORES, 128, w_total] src id or N_NODES (pad)
    si_gid: np.ndarray  # [N_CORES, 128, n_tiles] dst gid clipped (for si gather)
    flags: np.ndarray  # [N_CORES, 128, n_tiles] f32 1.0 where real dst with deg>0
    tile_col0: np.ndarray  # [n_tiles] starting col of each tile
    tile_D: np.ndarray  # [n_tiles] capacity of each tile


def build_schedule(edge_index: np.ndarray) -> Schedule:
    src = np.asarray(edge_index[0], dtype=np.int64)
    dst = np.asarray(edge_index[1], dtype=np.int64)
    order = np.argsort(dst, kind="stable")
    src_s = src[order]
    deg_all = np.bincount(dst, minlength=N_NODES).astype(np.int64)
    starts_all = np.concatenate([[0], np.cumsum(deg_all)])

    n_tiles = -(-DPC // 128)
    if n_tiles % 2:
        n_tiles += 1
    ntile_slots = n_tiles * 128

    # per-core degree-sorted dst order, padded with -1
    gids = np.full((N_CORES, ntile_slots), -1, np.int64)
    for c in range(N_CORES):
        degc = deg_all[c * DPC : (c + 1) * DPC]
        rank = np.argsort(degc, kind="stable")
        gids[c, :DPC] = c * DPC + rank

    deg_pad = np.concatenate([deg_all, [0]])
    gclip = np.where(gids >= 0, gids, N_NODES)
    degs = deg_pad[gclip].reshape(N_CORES, n_tiles, 128)
    tile_max = degs.max(axis=2).max(axis=0)  # [n_tiles] cross-core max deg

    # call plan over tile PAIRS: group pairs while ntc*D <= CALL_W
    pair_max = np.maximum(tile_max[0::2], tile_max[1::2])
    calls = []
    col = 0
    p0 = 0
    n_pairs = n_tiles // 2
    def rup2(x):
        return (int(x) + 1) // 2 * 2

    while p0 < n_pairs:
        D = max(2, rup2(pair_max[p0]))
        npair = 1
        while p0 + npair < n_pairs:
            nd = max(D, rup2(pair_max[p0 + npair]))
            if (npair + 1) * 2 * nd > CALL_W:
                break
            D = nd
            npair += 1
        calls.append((2 * p0, 2 * npair, D, col))
        col += 2 * npair * D
        p0 += npair
    w_total = col

    tile_col0 = np.zeros(n_tiles, np.int64)
    tile_D = np.zeros(n_tiles, np.int64)
    for (t0, ntc, D, col0) in calls:
        for tl in range(ntc):
            tile_col0[t0 + tl] = col0 + tl * D
            tile_D[t0 + tl] = D

    # slot_src: vectorized CSR -> padded-slot scatter
    slot_src = np.full((N_CORES, 128, w_total), N_NODES, np.int64)
    colstart_of_slot = tile_col0[
        np.arange(ntile_slots) // 128
    ]  # [ntile_slots] per (tile,partition)
    for c in range(N_CORES):
        g = gclip[c]
        ne = deg_pad[g]
        p_of_slot = np.arange(ntile_slots) % 128
        # flat positions in [128, w_total]: p*w_total + colstart + d
        base = p_of_slot * w_total + colstart_of_slot
        tot = int(ne.sum())
        pos = np.repeat(base, ne) + (
            np.arange(tot) - np.repeat(np.cumsum(ne) - ne, ne)
        )
        srcidx = np.repeat(starts_all[g], ne) + (
            np.arange(tot) - np.repeat(np.cumsum(ne) - ne, ne)
        )
        flat = slot_src[c].reshape(-1)
        flat[pos] = src_s[srcidx]

    si_gid = gclip.reshape(N_CORES, n_tiles, 128).transpose(0, 2, 1)
    flags = (
        ((gids >= 0) & (deg_pad[gclip] > 0))
        .reshape(N_CORES, n_tiles, 128)
        .transpose(0, 2, 1)
        .astype(np.float32)
    )
    flags = np.ascontiguousarray(flags)
    si_gid = np.ascontiguousarray(si_gid)

    return Schedule(
        n_tiles, w_total, calls, gids, slot_src, si_gid, flags, tile_col0, tile_D
    )


# ---------------------------------------------------------------- prog A
def build_progA(n_loc=DPC, in_c=IN_C, f=F):
    nc = bacc.Bacc("TRN2", target_bir_lowering=False, debug=False, num_devices=N_CORES)
    xT = nc.dram_tensor("xT", [in_c, n_loc], F16, kind="ExternalInput").ap()
    W = nc.dram_tensor("W", [in_c, f], F16, kind="ExternalInput").ap()
    bW = nc.dram_tensor("bW", [f, 1], F32, kind="ExternalInput").ap()
    As = nc.dram_tensor("As", [f, 2], F16, kind="ExternalInput").ap()
    whT = nc.dram_tensor("whT", [f, n_loc], F16, kind="ExternalOutput").ap()
    s = nc.dram_tensor("s", [2, n_loc], F32, kind="ExternalOutput").ap()

    with tile.TileContext(nc) as tc:
        with tc.tile_pool(name="sb", bufs=1) as pool, tc.tile_pool(
            name="ps", bufs=2, space="PSUM"
        ) as pps, tc.tile_pool(name="sb2", bufs=3) as pool2:
            xT_sb = pool.tile([in_c, n_loc], F16)
            nc.sync.dma_start(out=xT_sb[:], in_=xT[:, :])
            W_sb = pool.tile([in_c, f], F16)
            nc.sync.dma_start(out=W_sb[:], in_=W[:, :])
            bW_sb = pool.tile([f, 1], F32)
            nc.sync.dma_start(out=bW_sb[:], in_=bW[:, :])
            As_sb = pool.tile([f, 2], F16)
            nc.sync.dma_start(out=As_sb[:], in_=As[:, :])

            CH = 512
            for c0 in range(0, n_loc, CH):
                ch = min(CH, n_loc - c0)
                ps_w = pps.tile([f, CH], F32, space="PSUM")
                nc.tensor.matmul(
                    out=ps_w[:, :ch],
                    lhsT=W_sb[:],
                    rhs=xT_sb[:, c0 : c0 + ch],
                    start=True,
                    stop=True,
                )
                wh_sb = pool2.tile([f, CH], F16, tag="wh")
                nc.scalar.activation(
                    out=wh_sb[:, :ch],
                    in_=ps_w[:, :ch],
                    func=mybir.ActivationFunctionType.Identity,
                    bias=bW_sb[:],
                )
                nc.sync.dma_start(out=whT[:, c0 : c0 + ch], in_=wh_sb[:, :ch])
                ps_s = pps.tile([2, CH], F32, space="PSUM")
                nc.tensor.matmul(
                    out=ps_s[:, :ch],
                    lhsT=As_sb[:],
                    rhs=wh_sb[:, :ch],
                    start=True,
                    stop=True,
                )
                s_sb = pool2.tile([2, CH], F32, tag="ssb")
                nc.vector.tensor_copy(out=s_sb[:, :ch], in_=ps_s[:, :ch])
                nc.sync.dma_start(out=s[:, c0 : c0 + ch], in_=s_sb[:, :ch])
    nc.compile()
    return nc


# ---------------------------------------------------------------- prog B
def build_progB(sched: Schedule, f=F):
    NT = sched.n_tiles
    WTOT = sched.w_total
    NPAIR = NT // 2
    nc = bacc.Bacc("TRN2", target_bir_lowering=False, debug=False, num_devices=N_CORES)
    stream = nc.dram_tensor("stream", [128, WTOT * f], F16, kind="ExternalInput").ap()
    sj_d = nc.dram_tensor("sj", [128, WTOT], F32, kind="ExternalInput").ap()
    si_d = nc.dram_tensor("si", [128, NT], F32, kind="ExternalInput").ap()
    flags_d = nc.dram_tensor("flags", [128, NT], F32, kind="ExternalInput").ap()
    bA_d = nc.dram_tensor("bA", [128, 1], F32, kind="ExternalInput").ap()
    WnBD_d = nc.dram_tensor("WnBD", [128, 128], F16, kind="ExternalInput").ap()
    bWn_d = nc.dram_tensor("bWn", [128, 1], F32, kind="ExternalInput").ap()
    AsBD_d = nc.dram_tensor("AsBD", [128, 4], F16, kind="ExternalInput").ap()
    whnT = nc.dram_tensor("whnT", [128, NPAIR * 128], F16, kind="ExternalOutput").ap()
    sn = nc.dram_tensor("sn", [4, NPAIR * 128], F32, kind="ExternalOutput").ap()

    X = mybir.AxisListType.X
    AF = mybir.ActivationFunctionType
    OP = mybir.AluOpType
    MAXNTC = max(ntc for (_, ntc, _, _) in sched.calls)

    def v(ap, dims, off=0):
        return dataclasses.replace(
            ap,
            ap=[list(ap.ap[0])] + [list(d) for d in dims],
            offset=ap.offset + off,
        )

    with tile.TileContext(nc) as tc:
        with tc.tile_pool(name="const", bufs=1) as pc, tc.tile_pool(
            name="io", bufs=3
        ) as pio, tc.tile_pool(name="work", bufs=2) as pw, tc.tile_pool(
            name="ps", bufs=2, space="PSUM"
        ) as pps, tc.tile_pool(name="ps2", bufs=2, space="PSUM") as pps2, tc.tile_pool(
            name="ep", bufs=2
        ) as pep:
            sj_sb = pc.tile([128, WTOT], F32)
            nc.sync.dma_start(out=sj_sb[:], in_=sj_d[:, :])
            si_sb = pc.tile([128, NT], F32)
            nc.sync.dma_start(out=si_sb[:], in_=si_d[:, :])
            flags_sb = pc.tile([128, NT], F32)
            nc.sync.dma_start(out=flags_sb[:], in_=flags_d[:, :])
            bA_sb = pc.tile([128, 1], F32)
            nc.sync.dma_start(out=bA_sb[:], in_=bA_d[:, :])
            WnBD_sb = pc.tile([128, 128], F16)
            nc.sync.dma_start(out=WnBD_sb[:], in_=WnBD_d[:, :])
            bWn_sb = pc.tile([128, 1], F32)
            nc.sync.dma_start(out=bWn_sb[:], in_=bWn_d[:, :])
            AsBD_sb = pc.tile([128, 4], F16)
            nc.sync.dma_start(out=AsBD_sb[:], in_=AsBD_d[:, :])
            ident = pc.tile([128, 128], F16)
            make_identity(nc, ident[:])

            # epilogue flush state: stacked-pair h columns awaiting matmul
            state = {"hgrp": None, "k0": 0, "n": 0}

            def flush_pairs():
                if not state["n"]:
                    return
                hgrp = state["hgrp"]
                k0 = state["k0"]
                cols = state["n"] * 128
                ps_w = pps2.tile([128, FLUSH_PAIRS * 128], F32, tag="psw", space="PSUM")
                nc.tensor.matmul(
                    out=ps_w[:, :cols],
                    lhsT=WnBD_sb[:],
                    rhs=hgrp[:, :cols],
                    start=True,
                    stop=True,
                )
                whn_sb = pep.tile([128, FLUSH_PAIRS * 128], F16, tag="whn")
                nc.scalar.activation(
                    out=whn_sb[:, :cols],
                    in_=ps_w[:, :cols],
                    func=AF.Identity,
                    bias=bWn_sb[:],
                )
                nc.sync.dma_start(
                    out=whnT[:, k0 * 128 : k0 * 128 + cols], in_=whn_sb[:, :cols]
                )
                ps_s = pps2.tile([4, FLUSH_PAIRS * 128], F32, tag="pss", space="PSUM")
                nc.tensor.matmul(
                    out=ps_s[:, :cols],
                    lhsT=AsBD_sb[:],
                    rhs=whn_sb[:, :cols],
                    start=True,
                    stop=True,
                )
                s_sb = pep.tile([4, FLUSH_PAIRS * 128], F32, tag="ssb")
                nc.vector.tensor_copy(out=s_sb[:, :cols], in_=ps_s[:, :cols])
                nc.sync.dma_start(
                    out=sn[:, k0 * 128 : k0 * 128 + cols], in_=s_sb[:, :cols]
                )
                state["hgrp"] = None
                state["n"] = 0

            for (t0, ntc, D, col0) in sched.calls:
                W = ntc * D
                st = pio.tile([128, CALL_W * f], F16, tag="st")
                nc.sync.dma_start(
                    out=st[:, : W * f], in_=stream[:, col0 * f : (col0 + W) * f]
                )
                # e = leaky(s_j + s_i + bA); pads carry -1e30 inside sj
                epre = pw.tile([128, CALL_W], F32, tag="epre")
                nc.vector.tensor_tensor(
                    out=v(epre[:], [(D, ntc), (1, D)]),
                    in0=v(sj_sb[:], [(D, ntc), (1, D)], off=col0),
                    in1=si_sb[:, t0 : t0 + ntc].to_broadcast([128, ntc, D]),
                    op=OP.add,
                )
                e1 = pw.tile([128, CALL_W], F32, tag="e1")
                nc.scalar.activation(
                    out=e1[:, :W],
                    in_=epre[:, :W],
                    func=AF.Prelu,
                    bias=bA_sb[:],
                    alpha=ALPHA,
                )
                # segment softmax over slot axis
                m = pw.tile([128, MAXNTC], F32, tag="m")
                nc.vector.tensor_reduce(
                    out=m[:, :ntc],
                    in_=v(e1[:], [(D, ntc), (1, D)]),
                    axis=X,
                    op=OP.max,
                )
                nc.vector.tensor_tensor(
                    out=v(e1[:], [(D, ntc), (1, D)]),
                    in0=v(e1[:], [(D, ntc), (1, D)]),
                    in1=m[:, :ntc].to_broadcast([128, ntc, D]),
                    op=OP.subtract,
                )
                ex = pw.tile([128, CALL_W], F32, tag="ex")
                nc.scalar.activation(out=ex[:, :W], in_=e1[:, :W], func=AF.Exp)
                den = pw.tile([128, MAXNTC], F32, tag="den")
                nc.vector.tensor_reduce(
                    out=den[:, :ntc],
                    in_=v(ex[:], [(D, ntc), (1, D)]),
                    axis=X,
                    op=OP.add,
                )
                rnorm = pw.tile([128, MAXNTC], F32, tag="rnorm")
                nc.vector.reciprocal(out=rnorm[:, :ntc], in_=den[:, :ntc])
                nc.vector.tensor_tensor(
                    out=rnorm[:, :ntc],
                    in0=rnorm[:, :ntc],
                    in1=flags_sb[:, t0 : t0 + ntc],
                    op=OP.mult,
                )
                exn = pw.tile([128, CALL_W], F16, tag="exn")
                nc.vector.tensor_tensor(
                    out=v(exn[:], [(D, ntc), (1, D)]),
                    in0=v(ex[:], [(D, ntc), (1, D)]),
                    in1=rnorm[:, :ntc].to_broadcast([128, ntc, D]),
                    op=OP.mult,
                )
                # weighted messages in place over the stream tile (fp16,
                # feature-major: element (t, j, d) at offset t*f*D + j*D + d)
                nc.vector.tensor_tensor(
                    out=v(st[:], [(f * D, ntc), (D, f), (1, D)]),
                    in0=v(st[:], [(f * D, ntc), (D, f), (1, D)]),
                    in1=v(exn[:], [(D, ntc), (0, f), (1, D)]),
                    op=OP.mult,
                )
                hc = pw.tile([128, MAXNTC * f], F16, tag="hc")
                with nc.allow_low_precision(reason="fp16 segment sum, <=128 terms"):
                    nc.vector.tensor_reduce(
                        out=hc[:, : ntc * f],
                        in_=v(st[:], [(f * D, ntc), (D, f), (1, D)]),
                        axis=X,
                        op=OP.add,
                    )
                # epilogue per tile pair: transpose + leaky into the flush group
                for pr in range(ntc // 2):
                    kpair = (t0 + 2 * pr) // 2
                    ps_t = pps.tile([128, 128], F16, tag="pst", space="PSUM")
                    nc.tensor.transpose(
                        out=ps_t[:],
                        in_=hc[:, 2 * pr * f : (2 * pr + 2) * f],
                        identity=ident[:],
                    )
                    if state["n"] == 0:
                        state["hgrp"] = pep.tile(
                            [128, FLUSH_PAIRS * 128], F16, tag="hgrp", name="hgrp"
                        )
                        state["k0"] = kpair
                    j = state["n"]
                    nc.scalar.activation(
                        out=state["hgrp"][:, j * 128 : (j + 1) * 128],
                        in_=ps_t[:],
                        func=AF.Prelu,
                        alpha=ALPHA,
                    )
                    state["n"] += 1
                    if state["n"] == FLUSH_PAIRS:
                        flush_pairs()
            flush_pairs()
    nc.compile()
    return nc


# ---------------------------------------------------------------- driver
_cache = {}


def kernel(x, edge_index, W1, bW1, A1, bA1, W2, bW2, A2, bA2, Wfc, bfc):
    x = np.asarray(x, dtype=np.float32)
    edge_index = np.asarray(edge_index)
    W1 = np.asarray(W1, np.float32)
    bW1 = np.asarray(bW1, np.float32)
    A1 = np.asarray(A1, np.float32)
    bA1 = np.asarray(bA1, np.float32)
    W2 = np.asarray(W2, np.float32)
    bW2 = np.asarray(bW2, np.float32)
    A2 = np.asarray(A2, np.float32)
    bA2 = np.asarray(bA2, np.float32)
    Wfc = np.asarray(Wfc, np.float32)
    bfc = np.asarray(bfc, np.float32)

    sched = build_schedule(edge_index)
    cores = list(range(N_CORES))
    NT = sched.n_tiles
    NPAIR = NT // 2

    if "A" not in _cache:
        _cache["A"] = build_progA()
    ncA = _cache["A"]
    inA = []
    x16T = np.ascontiguousarray(x.T.astype(np.float16))
    W1_16 = W1.astype(np.float16)
    As1_16 = np.ascontiguousarray(
        np.concatenate([A1[:F], A1[F:]], axis=1).astype(np.float16)
    )
    for c in cores:
        inA.append(
            {
                "xT": np.ascontiguousarray(x16T[:, c * DPC : (c + 1) * DPC]),
                "W": W1_16,
                "bW": bW1.reshape(F, 1),
                "As": As1_16,
            }
        )
    resA = bass_utils.run_bass_kernel_spmd(ncA, inA, core_ids=cores)
    wh = np.concatenate([resA.results[c]["whT"].T for c in cores], axis=0)
    s_all = np.concatenate([resA.results[c]["s"] for c in cores], axis=1)
    si_full, sj_full = s_all[0], s_all[1]

    key = ("B", NT, sched.w_total, tuple(sched.calls))
    if key not in _cache:
        _cache[key] = build_progB(sched)
    ncB = _cache[key]

    def launch_B(wh_full, si_f, sj_f, bA, Wn, bWn, An):
        wh16 = np.concatenate(
            [wh_full.astype(np.float16), np.zeros((1, F), np.float16)], axis=0
        )
        sjpad = np.concatenate([sj_f, [np.float32(NEG_BIG)]]).astype(np.float32)
        sipad = np.concatenate([si_f, [np.float32(0.0)]]).astype(np.float32)
        WnBD = np.zeros((128, 128), np.float16)
        WnBD[:F, :F] = Wn
        WnBD[F:, F:] = Wn
        AsBD = np.zeros((128, 4), np.float16)
        AsBD[:F, 0:1] = An[:, 0:1]
        AsBD[:F, 1:2] = An[:, 1:2]
        AsBD[F:, 2:3] = An[:, 0:1]
        AsBD[F:, 3:4] = An[:, 1:2]
        bWn2 = np.concatenate([bWn.reshape(F), bWn.reshape(F)]).reshape(128, 1)
        inB = []
        for c in cores:
            ss = sched.slot_src[c]
            # feature-major stream: per call, element (t, j, d) at t*F*D+j*D+d
            stream = np.empty((128, sched.w_total * F), np.float16)
            for (t0, ntc, D, col0) in sched.calls:
                W = ntc * D
                blk = wh16[ss[:, col0 : col0 + W]].reshape(128, ntc, D, F)
                stream[:, col0 * F : (col0 + W) * F] = (
                    blk.transpose(0, 1, 3, 2).reshape(128, W * F)
                )
            inB.append(
                {
                    "stream": stream,
                    "sj": sjpad[ss],
                    "si": sipad[sched.si_gid[c]],
                    "flags": sched.flags[c],
                    "bA": np.full((128, 1), bA.reshape(-1)[0], np.float32),
                    "WnBD": WnBD,
                    "bWn": bWn2,
                    "AsBD": AsBD,
                }
            )
        res = bass_utils.run_bass_kernel_spmd(ncB, inB, core_ids=cores)
        whn = np.zeros((N_NODES, F), np.float32)
        sn_i = np.zeros(N_NODES, np.float32)
        sn_j = np.zeros(N_NODES, np.float32)
        for c in cores:
            gids = sched.gids[c]
            real = gids >= 0
            w = res.results[c]["whnT"].astype(np.float32).reshape(128, NPAIR, 128)
            snc = res.results[c]["sn"].reshape(4, NPAIR, 128)
            # tile 2k -> rows 0:64 of pair k; tile 2k+1 -> rows 64:128
            wA = w[:F].transpose(1, 2, 0)  # [NPAIR, 128, F] even tiles
            wB = w[F:].transpose(1, 2, 0)  # odd tiles
            wfull = np.empty((NT, 128, F), np.float32)
            wfull[0::2] = wA
            wfull[1::2] = wB
            sfull_i = np.empty((NT, 128), np.float32)
            sfull_j = np.empty((NT, 128), np.float32)
            sfull_i[0::2] = snc[0]
            sfull_i[1::2] = snc[2]
            sfull_j[0::2] = snc[1]
            sfull_j[1::2] = snc[3]
            whn[gids[real]] = wfull.reshape(NT * 128, F)[real]
            sn_i[gids[real]] = sfull_i.reshape(-1)[real]
            sn_j[gids[real]] = sfull_j.reshape(-1)[real]
        return whn, sn_i, sn_j

    As2 = np.ascontiguousarray(np.concatenate([A2[:F], A2[F:]], axis=1))
    wh2, si2, sj2 = launch_B(wh, si_full, sj_full, bA1, W2, bW2, As2)
    out, _, _ = launch_B(wh2, si2, sj2, bA2, Wfc, bfc, np.zeros((F, 2), np.float32))
    return out.astype(np.float32)


# revision 24
# speedup vs baseline: 11.8544x; 1.1068x over previous
"""GAT (2-layer) on 8 NeuronCores — Bass/Tile kernel.

Strategy (dst-sharded graph parallel, host-expanded dense streams):
  - Each core owns 12500 destination nodes, degree-sorted into 128-dst
    tiles; tiles are paired and grouped into calls with a shared
    per-call slot capacity D (cross-core max), giving a dense
    [128 dst x D slot] layout per tile.
  - Launch A: per-core Wh1^T = (x W1 + b)^T and attention scalars
    s_i/s_j (all model FLOPs on device).
  - Host pre/re-pack (pure indexing of device-computed values): expands
    the per-edge source stream  stream16[p, col, f] = Wh[src] (fp16),
    sj_slot[p, col] = s_j[src] (f32, -1e30 at pad slots), si per tile,
    zero-degree flags.  No arithmetic on features happens on host.
  - Launch B (x2, one per GAT layer): streams the dense fp16 tables at
    line rate (plain dma_start, no gathers), computes masked segment
    softmax over the slot axis, alpha-weighted message sum (fp16
    multiply in place, f32 accumulate), leaky-relu, and the epilogue
    matmul with the next layer's weights (block-diagonal pair trick)
    -> next-layer Wh^T + attention scalars (or final fc output).
"""

import dataclasses
import numpy as np

import concourse.bacc as bacc
import concourse.tile as tile
from concourse import bass, mybir, bass_utils
from concourse.masks import make_identity

F32 = mybir.dt.float32
F16 = mybir.dt.float16

N_NODES = 100000
N_CORES = 8
DPC = N_NODES // N_CORES
F = 64
IN_C = 128
NEG_BIG = -1.0e30
ALPHA = 0.2
CALL_W = 384  # max slot-columns per call chunk
FLUSH_PAIRS = 4  # tile-pairs per epilogue matmul (512 psum cols)


@dataclasses.dataclass
class Schedule:
    n_tiles: int  # tiles per core (even)
    w_total: int  # total slot columns
    calls: list  # (t0, ntc, D, col0) ; ntc even
    gids: np.ndarray  # [N_CORES, n_tiles*128] global dst id or -1
    slot_src: np.ndarray  # [N_CORES, 128, w_total] src id or N_NODES (pad)
    si_gid: np.ndarray  # [N_CORES, 128, n_tiles] dst gid clipped (for si gather)
    flags: np.ndarray  # [N_CORES, 128, n_tiles] f32 1.0 where real dst with deg>0
    tile_col0: np.ndarray  # [n_tiles] starting col of each tile
    tile_D: np.ndarray  # [n_tiles] capacity of each tile


def build_schedule(edge_index: np.ndarray) -> Schedule:
    src = np.asarray(edge_index[0], dtype=np.int64)
    dst = np.asarray(edge_index[1], dtype=np.int64)
    order = np.argsort(dst, kind="stable")
    src_s = src[order]
    deg_all = np.bincount(dst, minlength=N_NODES).astype(np.int64)
    starts_all = np.concatenate([[0], np.cumsum(deg_all)])

    n_tiles = -(-DPC // 128)
    if n_tiles % 2:
        n_tiles += 1
    ntile_slots = n_tiles * 128

    # per-core degree-sorted dst order, padded with -1
    gids = np.full((N_CORES, ntile_slots), -1, np.int64)
    for c in range(N_CORES):
        degc = deg_all[c * DPC : (c + 1) * DPC]
        rank = np.argsort(degc, kind="stable")
        gids[c, :DPC] = c * DPC + rank

    deg_pad = np.concatenate([deg_all, [0]])
    gclip = np.where(gids >= 0, gids, N_NODES)
    degs = deg_pad[gclip].reshape(N_CORES, n_tiles, 128)
    tile_max = degs.max(axis=2).max(axis=0)  # [n_tiles] cross-core max deg

    # call plan over tile PAIRS: group pairs while ntc*D <= CALL_W
    pair_max = np.maximum(tile_max[0::2], tile_max[1::2])
    calls = []
    col = 0
    p0 = 0
    n_pairs = n_tiles // 2
    def rup4(x):
        return (int(x) + 3) // 4 * 4

    while p0 < n_pairs:
        D = max(4, rup4(pair_max[p0]))
        npair = 1
        while p0 + npair < n_pairs:
            nd = max(D, rup4(pair_max[p0 + npair]))
            if (npair + 1) * 2 * nd > CALL_W:
                break
            D = nd
            npair += 1
        calls.append((2 * p0, 2 * npair, D, col))
        col += 2 * npair * D
        p0 += npair
    w_total = col

    tile_col0 = np.zeros(n_tiles, np.int64)
    tile_D = np.zeros(n_tiles, np.int64)
    for (t0, ntc, D, col0) in calls:
        for tl in range(ntc):
            tile_col0[t0 + tl] = col0 + tl * D
            tile_D[t0 + tl] = D

    # slot_src: vectorized CSR -> padded-slot scatter
    slot_src = np.full((N_CORES, 128, w_total), N_NODES, np.int64)
    colstart_of_slot = tile_col0[
        np.arange(ntile_slots) // 128
    ]  # [ntile_slots] per (tile,partition)
    for c in range(N_CORES):
        g = gclip[c]
        ne = deg_pad[g]
        p_of_slot = np.arange(ntile_slots) % 128
        # flat positions in [128, w_total]: p*w_total + colstart + d
        base = p_of_slot * w_total + colstart_of_slot
        tot = int(ne.sum())
        pos = np.repeat(base, ne) + (
            np.arange(tot) - np.repeat(np.cumsum(ne) - ne, ne)
        )
        srcidx = np.repeat(starts_all[g], ne) + (
            np.arange(tot) - np.repeat(np.cumsum(ne) - ne, ne)
        )
        flat = slot_src[c].reshape(-1)
        flat[pos] = src_s[srcidx]

    si_gid = gclip.reshape(N_CORES, n_tiles, 128).transpose(0, 2, 1)
    flags = (
        ((gids >= 0) & (deg_pad[gclip] > 0))
        .reshape(N_CORES, n_tiles, 128)
        .transpose(0, 2, 1)
        .astype(np.float32)
    )
    flags = np.ascontiguousarray(flags)
    si_gid = np.ascontiguousarray(si_gid)

    return Schedule(
        n_tiles, w_total, calls, gids, slot_src, si_gid, flags, tile_col0, tile_D
    )


# ---------------------------------------------------------------- prog A
def build_progA(n_loc=DPC, in_c=IN_C, f=F):
    """whs[0:64] = (x W + bW)^T fp16 ; whs[64] = s_i ; whs[65] = s_j.

    Uses an augmented weight Waug = [W | W@A_i | W@A_j] (built on device)
    so each 512-column chunk is one matmul + one activation:
      x (W As) + bW As == ((x W + bW) As).
    """
    AF = mybir.ActivationFunctionType
    nc = bacc.Bacc("TRN2", target_bir_lowering=False, debug=False, num_devices=N_CORES)
    xT = nc.dram_tensor("xT", [in_c, n_loc], F16, kind="ExternalInput").ap()
    W = nc.dram_tensor("W", [in_c, f], F16, kind="ExternalInput").ap()
    bW = nc.dram_tensor("bW", [f, 1], F32, kind="ExternalInput").ap()
    As = nc.dram_tensor("As", [f, 2], F16, kind="ExternalInput").ap()
    whs = nc.dram_tensor("whs", [f + 2, n_loc], F16, kind="ExternalOutput").ap()

    CH = 512
    BATCH = 4

    with tile.TileContext(nc) as tc:
        with tc.tile_pool(name="sb", bufs=1) as pool, tc.tile_pool(
            name="ps", bufs=3, space="PSUM"
        ) as pps, tc.tile_pool(name="sb2", bufs=3) as pool2:
            xT_sb = pool.tile([in_c, n_loc], F16)
            nc.sync.dma_start(out=xT_sb[:], in_=xT[:, :])
            W_sb = pool.tile([in_c, f], F16)
            nc.sync.dma_start(out=W_sb[:], in_=W[:, :])
            bW_sb = pool.tile([f, 1], F32)
            nc.sync.dma_start(out=bW_sb[:], in_=bW[:, :])
            As_sb = pool.tile([f, 2], F16)
            nc.sync.dma_start(out=As_sb[:], in_=As[:, :])
            ident = pool.tile([128, 128], F16)
            make_identity(nc, ident[:])

            # Waug = [W | W@As] built on device
            Waug = pool.tile([in_c, f + 2], F16)
            nc.vector.tensor_copy(out=Waug[:, :f], in_=W_sb[:])
            ps_wt = pps.tile([f, 128], F16, space="PSUM", bufs=1)
            nc.tensor.transpose(out=ps_wt[:], in_=W_sb[:], identity=ident[:])
            WT_sb = pool.tile([f, 128], F16)
            nc.scalar.activation(out=WT_sb[:], in_=ps_wt[:], func=AF.Identity)
            ps_was = pps.tile([2, 128], F32, space="PSUM", bufs=1)
            nc.tensor.matmul(
                out=ps_was[:], lhsT=As_sb[:], rhs=WT_sb[:], start=True, stop=True
            )
            WAsT_sb = pool.tile([2, 128], F16)
            nc.scalar.activation(out=WAsT_sb[:], in_=ps_was[:], func=AF.Identity)
            ps_was2 = pps.tile([128, 2], F16, space="PSUM", bufs=1)
            nc.tensor.transpose(
                out=ps_was2[:], in_=WAsT_sb[:], identity=ident[:2, :2]
            )
            nc.scalar.activation(out=Waug[:, f : f + 2], in_=ps_was2[:], func=AF.Identity)

            # baug = [bW ; bW@As]
            baug = pool.tile([f + 2, 1], F32)
            nc.vector.tensor_copy(out=baug[:f], in_=bW_sb[:])
            bW16 = pool.tile([f, 1], F16)
            nc.vector.tensor_copy(out=bW16[:], in_=bW_sb[:])
            ps_bas = pps.tile([2, 1], F32, space="PSUM", bufs=1)
            nc.tensor.matmul(
                out=ps_bas[:], lhsT=As_sb[:], rhs=bW16[:], start=True, stop=True
            )
            nc.vector.tensor_copy(out=baug[f : f + 2], in_=ps_bas[:])

            for b0 in range(0, n_loc, CH * BATCH):
                bw = min(CH * BATCH, n_loc - b0)
                out_sb = pool2.tile([f + 2, CH * BATCH], F16, tag="out")
                for c0 in range(b0, b0 + bw, CH):
                    ch = min(CH, b0 + bw - c0)
                    ps_w = pps.tile([f + 2, CH], F32, tag="psw", space="PSUM")
                    nc.tensor.matmul(
                        out=ps_w[:, :ch],
                        lhsT=Waug[:],
                        rhs=xT_sb[:, c0 : c0 + ch],
                        start=True,
                        stop=True,
                    )
                    nc.scalar.activation(
                        out=out_sb[:, c0 - b0 : c0 - b0 + ch],
                        in_=ps_w[:, :ch],
                        func=AF.Identity,
                        bias=baug[:],
                    )
                nc.sync.dma_start(out=whs[:, b0 : b0 + bw], in_=out_sb[:, :bw])
    nc.compile()
    return nc


# ---------------------------------------------------------------- prog B
def build_progB(sched: Schedule, f=F):
    NT = sched.n_tiles
    WTOT = sched.w_total
    NPAIR = NT // 2
    nc = bacc.Bacc("TRN2", target_bir_lowering=False, debug=False, num_devices=N_CORES)
    stream = nc.dram_tensor("stream", [128, WTOT * f], F16, kind="ExternalInput").ap()
    sj_d = nc.dram_tensor("sj", [128, WTOT], F32, kind="ExternalInput").ap()
    si_d = nc.dram_tensor("si", [128, NT], F32, kind="ExternalInput").ap()
    flags_d = nc.dram_tensor("flags", [128, NT], F32, kind="ExternalInput").ap()
    bA_d = nc.dram_tensor("bA", [128, 1], F32, kind="ExternalInput").ap()
    WnBD_d = nc.dram_tensor("WnBD", [128, 128], F16, kind="ExternalInput").ap()
    bWn_d = nc.dram_tensor("bWn", [128, 1], F32, kind="ExternalInput").ap()
    AsBD_d = nc.dram_tensor("AsBD", [128, 4], F16, kind="ExternalInput").ap()
    whnT = nc.dram_tensor("whnT", [128, NPAIR * 128], F16, kind="ExternalOutput").ap()
    sn = nc.dram_tensor("sn", [4, NPAIR * 128], F32, kind="ExternalOutput").ap()

    X = mybir.AxisListType.X
    AF = mybir.ActivationFunctionType
    OP = mybir.AluOpType
    MAXNTC = max(ntc for (_, ntc, _, _) in sched.calls)

    def v(ap, dims, off=0):
        return dataclasses.replace(
            ap,
            ap=[list(ap.ap[0])] + [list(d) for d in dims],
            offset=ap.offset + off,
        )

    with tile.TileContext(nc) as tc:
        with tc.tile_pool(name="const", bufs=1) as pc, tc.tile_pool(
            name="io", bufs=3
        ) as pio, tc.tile_pool(name="work", bufs=2) as pw, tc.tile_pool(
            name="ps", bufs=2, space="PSUM"
        ) as pps, tc.tile_pool(name="ps2", bufs=2, space="PSUM") as pps2, tc.tile_pool(
            name="ep", bufs=2
        ) as pep:
            sj_sb = pc.tile([128, WTOT], F32)
            nc.sync.dma_start(out=sj_sb[:], in_=sj_d[:, :])
            si_sb = pc.tile([128, NT], F32)
            nc.sync.dma_start(out=si_sb[:], in_=si_d[:, :])
            flags_sb = pc.tile([128, NT], F32)
            nc.sync.dma_start(out=flags_sb[:], in_=flags_d[:, :])
            bA_sb = pc.tile([128, 1], F32)
            nc.sync.dma_start(out=bA_sb[:], in_=bA_d[:, :])
            WnBD_sb = pc.tile([128, 128], F16)
            nc.sync.dma_start(out=WnBD_sb[:], in_=WnBD_d[:, :])
            bWn_sb = pc.tile([128, 1], F32)
            nc.sync.dma_start(out=bWn_sb[:], in_=bWn_d[:, :])
            AsBD_sb = pc.tile([128, 4], F16)
            nc.sync.dma_start(out=AsBD_sb[:], in_=AsBD_d[:, :])
            ident = pc.tile([128, 128], F16)
            make_identity(nc, ident[:])

            # epilogue flush state: stacked-pair h columns awaiting matmul
            state = {"hgrp": None, "k0": 0, "n": 0}

            def flush_pairs():
                if not state["n"]:
                    return
                hgrp = state["hgrp"]
                k0 = state["k0"]
                cols = state["n"] * 128
                ps_w = pps2.tile([128, FLUSH_PAIRS * 128], F32, tag="psw", space="PSUM")
                nc.tensor.matmul(
                    out=ps_w[:, :cols],
                    lhsT=WnBD_sb[:],
                    rhs=hgrp[:, :cols],
                    start=True,
                    stop=True,
                )
                whn_sb = pep.tile([128, FLUSH_PAIRS * 128], F16, tag="whn")
                nc.scalar.activation(
                    out=whn_sb[:, :cols],
                    in_=ps_w[:, :cols],
                    func=AF.Identity,
                    bias=bWn_sb[:],
                )
                nc.sync.dma_start(
                    out=whnT[:, k0 * 128 : k0 * 128 + cols], in_=whn_sb[:, :cols]
                )
                ps_s = pps2.tile([4, FLUSH_PAIRS * 128], F32, tag="pss", space="PSUM")
                nc.tensor.matmul(
                    out=ps_s[:, :cols],
                    lhsT=AsBD_sb[:],
                    rhs=whn_sb[:, :cols],
                    start=True,
                    stop=True,
                )
                s_sb = pep.tile([4, FLUSH_PAIRS * 128], F32, tag="ssb")
                nc.scalar.activation(
                    out=s_sb[:, :cols], in_=ps_s[:, :cols], func=AF.Identity
                )
                nc.sync.dma_start(
                    out=sn[:, k0 * 128 : k0 * 128 + cols], in_=s_sb[:, :cols]
                )
                state["hgrp"] = None
                state["n"] = 0

            for (t0, ntc, D, col0) in sched.calls:
                W = ntc * D
                st = pio.tile([128, CALL_W * f], F16, tag="st")
                nc.sync.dma_start(
                    out=st[:, : W * f], in_=stream[:, col0 * f : (col0 + W) * f]
                )
                # e = leaky(s_j + s_i + bA); pads carry -1e30 inside sj
                epre = pw.tile([128, CALL_W], F32, tag="epre")
                nc.vector.tensor_tensor(
                    out=v(epre[:], [(D, ntc), (1, D)]),
                    in0=v(sj_sb[:], [(D, ntc), (1, D)], off=col0),
                    in1=si_sb[:, t0 : t0 + ntc].to_broadcast([128, ntc, D]),
                    op=OP.add,
                )
                e1 = pw.tile([128, CALL_W], F32, tag="e1")
                nc.scalar.activation(
                    out=e1[:, :W],
                    in_=epre[:, :W],
                    func=AF.Prelu,
                    bias=bA_sb[:],
                    alpha=ALPHA,
                )
                # segment softmax over slot axis. No max-subtraction: the
                # shift cancels in exp(e)/sum(exp(e)) and |e| <= ~20 here;
                # +1e-30 guards all-pad (phantom) rows against 1/0.
                ex = pw.tile([128, CALL_W], F32, tag="ex")
                nc.scalar.activation(out=ex[:, :W], in_=e1[:, :W], func=AF.Exp)
                den = pw.tile([128, MAXNTC], F32, tag="den")
                nc.vector.tensor_reduce(
                    out=den[:, :ntc],
                    in_=v(ex[:], [(D, ntc), (1, D)]),
                    axis=X,
                    op=OP.add,
                )
                dene = pw.tile([128, MAXNTC], F32, tag="dene")
                nc.vector.tensor_scalar(
                    out=dene[:, :ntc],
                    in0=den[:, :ntc],
                    scalar1=1e-30,
                    scalar2=None,
                    op0=OP.add,
                )
                rnorm = pw.tile([128, MAXNTC], F32, tag="rnorm")
                nc.vector.reciprocal(out=rnorm[:, :ntc], in_=dene[:, :ntc])
                nc.vector.tensor_tensor(
                    out=rnorm[:, :ntc],
                    in0=rnorm[:, :ntc],
                    in1=flags_sb[:, t0 : t0 + ntc],
                    op=OP.mult,
                )
                exn = pw.tile([128, CALL_W], F16, tag="exn")
                nc.vector.tensor_tensor(
                    out=v(exn[:], [(D, ntc), (1, D)]),
                    in0=v(ex[:], [(D, ntc), (1, D)]),
                    in1=rnorm[:, :ntc].to_broadcast([128, ntc, D]),
                    op=OP.mult,
                )
                # weighted messages in place over the stream tile (fp16,
                # feature-major: element (t, j, d) at offset t*f*D + j*D + d)
                nc.vector.tensor_tensor(
                    out=v(st[:], [(f * D, ntc), (D, f), (1, D)]),
                    in0=v(st[:], [(f * D, ntc), (D, f), (1, D)]),
                    in1=v(exn[:], [(D, ntc), (0, f), (1, D)]),
                    op=OP.mult,
                )
                # fold D -> D/2 with a 2x-mode tensor_tensor add (D % 4 == 0
                # so both halves stay pair-aligned), then 1x-mode reduce
                D2 = D // 2
                with nc.allow_low_precision(reason="fp16 segment sum, <=128 terms"):
                    nc.vector.tensor_tensor(
                        out=v(st[:], [(f * D, ntc), (D, f), (1, D2)]),
                        in0=v(st[:], [(f * D, ntc), (D, f), (1, D2)]),
                        in1=v(st[:], [(f * D, ntc), (D, f), (1, D2)], off=D2),
                        op=OP.add,
                    )
                    hc = pw.tile([128, MAXNTC * f], F16, tag="hc")
                    nc.vector.tensor_reduce(
                        out=hc[:, : ntc * f],
                        in_=v(st[:], [(f * D, ntc), (D, f), (1, D2)]),
                        axis=X,
                        op=OP.add,
                    )
                # epilogue per tile pair: transpose + leaky into the flush group
                for pr in range(ntc // 2):
                    kpair = (t0 + 2 * pr) // 2
                    ps_t = pps.tile([128, 128], F16, tag="pst", space="PSUM")
                    nc.tensor.transpose(
                        out=ps_t[:],
                        in_=hc[:, 2 * pr * f : (2 * pr + 2) * f],
                        identity=ident[:],
                    )
                    if state["n"] == 0:
                        state["hgrp"] = pep.tile(
                            [128, FLUSH_PAIRS * 128], F16, tag="hgrp", name="hgrp"
                        )
                        state["k0"] = kpair
                    j = state["n"]
                    nc.scalar.activation(
                        out=state["hgrp"][:, j * 128 : (j + 1) * 128],
                        in_=ps_t[:],
                        func=AF.Prelu,
                        alpha=ALPHA,
                    )
                    state["n"] += 1
                    if state["n"] == FLUSH_PAIRS:
                        flush_pairs()
            flush_pairs()
    nc.compile()
    return nc


# ---------------------------------------------------------------- driver
_cache = {}


def kernel(x, edge_index, W1, bW1, A1, bA1, W2, bW2, A2, bA2, Wfc, bfc):
    x = np.asarray(x, dtype=np.float32)
    edge_index = np.asarray(edge_index)
    W1 = np.asarray(W1, np.float32)
    bW1 = np.asarray(bW1, np.float32)
    A1 = np.asarray(A1, np.float32)
    bA1 = np.asarray(bA1, np.float32)
    W2 = np.asarray(W2, np.float32)
    bW2 = np.asarray(bW2, np.float32)
    A2 = np.asarray(A2, np.float32)
    bA2 = np.asarray(bA2, np.float32)
    Wfc = np.asarray(Wfc, np.float32)
    bfc = np.asarray(bfc, np.float32)

    sched = build_schedule(edge_index)
    cores = list(range(N_CORES))
    NT = sched.n_tiles
    NPAIR = NT // 2

    if "A" not in _cache:
        _cache["A"] = build_progA()
    ncA = _cache["A"]
    inA = []
    x16T = np.ascontiguousarray(x.T.astype(np.float16))
    W1_16 = W1.astype(np.float16)
    As1_16 = np.ascontiguousarray(
        np.concatenate([A1[:F], A1[F:]], axis=1).astype(np.float16)
    )
    for c in cores:
        inA.append(
            {
                "xT": np.ascontiguousarray(x16T[:, c * DPC : (c + 1) * DPC]),
                "W": W1_16,
                "bW": bW1.reshape(F, 1),
                "As": As1_16,
            }
        )
    resA = bass_utils.run_bass_kernel_spmd(ncA, inA, core_ids=cores)
    whs = np.concatenate([resA.results[c]["whs"] for c in cores], axis=1)
    wh = np.ascontiguousarray(whs[:F].T)
    si_full = whs[F].astype(np.float32)
    sj_full = whs[F + 1].astype(np.float32)

    key = ("B", NT, sched.w_total, tuple(sched.calls))
    if key not in _cache:
        _cache[key] = build_progB(sched)
    ncB = _cache[key]

    def launch_B(wh_full, si_f, sj_f, bA, Wn, bWn, An):
        wh16 = np.concatenate(
            [wh_full.astype(np.float16), np.zeros((1, F), np.float16)], axis=0
        )
        sjpad = np.concatenate([sj_f, [np.float32(NEG_BIG)]]).astype(np.float32)
        sipad = np.concatenate([si_f, [np.float32(0.0)]]).astype(np.float32)
        WnBD = np.zeros((128, 128), np.float16)
        WnBD[:F, :F] = Wn
        WnBD[F:, F:] = Wn
        AsBD = np.zeros((128, 4), np.float16)
        AsBD[:F, 0:1] = An[:, 0:1]
        AsBD[:F, 1:2] = An[:, 1:2]
        AsBD[F:, 2:3] = An[:, 0:1]
        AsBD[F:, 3:4] = An[:, 1:2]
        bWn2 = np.concatenate([bWn.reshape(F), bWn.reshape(F)]).reshape(128, 1)
        inB = []
        for c in cores:
            ss = sched.slot_src[c]
            # feature-major stream: per call, element (t, j, d) at t*F*D+j*D+d
            stream = np.empty((128, sched.w_total * F), np.float16)
            for (t0, ntc, D, col0) in sched.calls:
                W = ntc * D
                blk = wh16[ss[:, col0 : col0 + W]].reshape(128, ntc, D, F)
                stream[:, col0 * F : (col0 + W) * F] = (
                    blk.transpose(0, 1, 3, 2).reshape(128, W * F)
                )
            inB.append(
                {
                    "stream": stream,
                    "sj": sjpad[ss],
                    "si": sipad[sched.si_gid[c]],
                    "flags": sched.flags[c],
                    "bA": np.full((128, 1), bA.reshape(-1)[0], np.float32),
                    "WnBD": WnBD,
                    "bWn": bWn2,
                    "AsBD": AsBD,
                }
            )
        res = bass_utils.run_bass_kernel_spmd(ncB, inB, core_ids=cores)
        whn = np.zeros((N_NODES, F), np.float32)
        sn_i = np.zeros(N_NODES, np.float32)
        sn_j = np.zeros(N_NODES, np.float32)
        for c in cores:
            gids = sched.gids[c]
            real = gids >= 0
            w = res.results[c]["whnT"].astype(np.float32).reshape(128, NPAIR, 128)
            snc = res.results[c]["sn"].reshape(4, NPAIR, 128)
            # tile 2k -> rows 0:64 of pair k; tile 2k+1 -> rows 64:128
            wA = w[:F].transpose(1, 2, 0)  # [NPAIR, 128, F] even tiles
            wB = w[F:].transpose(1, 2, 0)  # odd tiles
            wfull = np.empty((NT, 128, F), np.float32)
            wfull[0::2] = wA
            wfull[1::2] = wB
            sfull_i = np.empty((NT, 128), np.float32)
            sfull_j = np.empty((NT, 128), np.float32)
            sfull_i[0::2] = snc[0]
            sfull_i[1::2] = snc[2]
            sfull_j[0::2] = snc[1]
            sfull_j[1::2] = snc[3]
            whn[gids[real]] = wfull.reshape(NT * 128, F)[real]
            sn_i[gids[real]] = sfull_i.reshape(-1)[real]
            sn_j[gids[real]] = sfull_j.reshape(-1)[real]
        return whn, sn_i, sn_j

    As2 = np.ascontiguousarray(np.concatenate([A2[:F], A2[F:]], axis=1))
    wh2, si2, sj2 = launch_B(wh, si_full, sj_full, bA1, W2, bW2, As2)
    out, _, _ = launch_B(wh2, si2, sj2, bA2, Wfc, bfc, np.zeros((F, 2), np.float32))
    return out.astype(np.float32)
